# revision 12
# baseline (speedup 1.0000x reference)
"""Sparse-attention distance-mask kernel for Trainium2 (8 NeuronCores).

Reference computation (per batch b):
    pos      = multi-hot of 4 tree-position ids over 512 nodes   [seq, 512]
    dist     = s_i + s_j - 2 * pos @ pos.T          (L1 dist of binary vecs)
    attn     = max(dist_top, dist_left)
    out      = attn + padding_dist * max(pad_i, pad_j)

Kernel strategy:
  - Data-parallel over batch: core c computes batch c (b == n_cores == 8).
  - The whole distance-with-padding map folds into ONE augmented GEMM:
        dist + pad_mat = (-2 pos_i) . pos_j  +  A_i * 1  +  1 * B_j
                          + pad_i * (-p pad_j)
    with A_i = s_i + p*pad_i, B_j = s_j + p*pad_j, p = padding_dist.
    lhsT = [-2 pos^T ; A ; 1 ; pad], rhs = [pos^T ; 1 ; B ; -p*pad], K = 515.
  - pos entries are {0, 1, -2} -> exact in fp8(e4m3), halving input DMA; the
    pos block runs as 4 plain fp8 matmul passes (K=128 each; DoubleRow
    measured slower here since the N=512 moving stream dominates).  The 3
    aug rows run as one bf16 K=3 pass (all values integers <= 256 ->
    bf16-exact for p=100).  PSUM accumulates fp32 -> bit-exact result.
  - The distance map is symmetric, so only 12 of 16 [128,512] output blocks
    are computed (blocks entirely below the diagonal are mirrored on host).
  - Epilogue per block: ACT copies top-PSUM to SBUF, DVE maxes left-PSUM in,
    row-merged DMA stores.
"""

import os

import ml_dtypes
import numpy as np

B, SEQ, DEPTH = 8, 1024, 4
TN = 512          # TOTAL_NODE
AUG = 3
N_CORES = 8
MB, NB = SEQ // 128, SEQ // 512   # 8 x 2 grid of [128, 512] blocks
# skip blocks entirely below the diagonal (mirrored on host)
BLOCKS = [(mb, nb) for mb in range(MB) for nb in range(NB)
          if not (nb == 0 and mb >= 4)]

_NC_CACHE = {}
LAST_RESULTS = None

_POS_NAMES = ("lhs_top", "rhs_top", "lhs_left", "rhs_left")


def _build_nc():
    import concourse.mybir as mybir
    from concourse import bacc
    from concourse.tile import TileContext

    nc = bacc.Bacc()
    dram = {}
    for name in _POS_NAMES:
        dram[name] = nc.dram_tensor(
            name, [128, 4 * SEQ], mybir.dt.float8e4, kind="ExternalInput"
        )
    dram["augs"] = nc.dram_tensor(
        "augs", [AUG, 4 * SEQ], mybir.dt.bfloat16, kind="ExternalInput"
    )
    out = nc.dram_tensor("out", [SEQ, SEQ], mybir.dt.float32, kind="ExternalOutput")

    with TileContext(nc) as tc:
        with (
            tc.tile_pool(name="w", bufs=1) as wpool,
            tc.tile_pool(name="ps", bufs=2, space="PSUM") as ppool,
            tc.tile_pool(name="ep", bufs=1) as epool,
        ):
            sb = {}
            # one DMA per tensor, top-mask tensors first so PE starts early
            for name in _POS_NAMES:
                w = wpool.tile([128, 4 * SEQ], mybir.dt.float8e4,
                               tag=name, name=name)
                sb[name] = w
            augs = wpool.tile([AUG, 4 * SEQ], mybir.dt.bfloat16,
                              tag="augs", name="augs")
            for name in ("lhs_top", "rhs_top"):
                nc.sync.dma_start(out=sb[name][:, :], in_=dram[name][:, :])
            nc.sync.dma_start(out=augs[:, :], in_=dram["augs"][:, :])
            for name in ("lhs_left", "rhs_left"):
                nc.sync.dma_start(out=sb[name][:, :], in_=dram[name][:, :])

            # cp tiles: mb 0-3 hold a full [128,1024] row; mb 4-7 (upper
            # halves only) are packed into one [128, 4*512] tile so the
            # whole lower-right quadrant stores with a single DMA.
            cps = {mb: epool.tile([128, SEQ], mybir.dt.float32,
                                  tag=f"cp{mb}", name=f"cp{mb}")
                   for mb in range(4)}
            cphi = epool.tile([128, 4 * 512], mybir.dt.float32,
                              tag="cphi", name="cphi")

            def cp_slice(mb, nb):
                if mb < 4:
                    return cps[mb][:, nb * 512:(nb + 1) * 512]
                return cphi[:, (mb - 4) * 512:(mb - 3) * 512]

            def gemm(psum, lname, rname, aug_l, aug_r, mb, nb):
                ms = slice(mb * 128, (mb + 1) * 128)
                ns = slice(nb * 512, (nb + 1) * 512)
                for kt in range(4):
                    nc.tensor.matmul(
                        psum[:, :],
                        lhsT=sb[lname][:, kt * SEQ + mb * 128:
                                       kt * SEQ + mb * 128 + 128],
                        rhs=sb[rname][:, kt * SEQ + nb * 512:
                                      kt * SEQ + nb * 512 + 512],
                        start=(kt == 0),
                        stop=False,
                    )
                nc.tensor.matmul(
                    psum[:, :],
                    lhsT=augs[:, aug_l * SEQ + mb * 128:
                              aug_l * SEQ + mb * 128 + 128],
                    rhs=augs[:, aug_r * SEQ + nb * 512:
                             aug_r * SEQ + nb * 512 + 512],
                    start=False,
                    stop=True,
                    skip_group_check=True,
                )

            # lower-right quadrant blocks first so their (single) store DMA
            # overlaps the remaining compute
            ordered = ([(mb, 1) for mb in range(4, 8)] +
                       [(mb, nb) for mb in range(4) for nb in range(2)])

            # Phase A: top-mask GEMMs -> ACT copy into cp
            for mb, nb in ordered:
                ps_t = ppool.tile([128, 512], mybir.dt.float32, tag="pt",
                                  name=f"pt{mb}_{nb}")
                gemm(ps_t, "lhs_top", "rhs_top", 0, 1, mb, nb)
                nc.scalar.copy(cp_slice(mb, nb), ps_t[:, :])

            # Phase B: left-mask GEMMs -> DVE max -> store
            for mb, nb in ordered:
                ps_l = ppool.tile([128, 512], mybir.dt.float32, tag="pl",
                                  name=f"pl{mb}_{nb}")
                gemm(ps_l, "lhs_left", "rhs_left", 2, 3, mb, nb)
                sl = cp_slice(mb, nb)
                nc.vector.tensor_max(sl, sl, ps_l[:, :])
                if mb == 7:
                    # one DMA for the whole lower-right quadrant:
                    # DRAM [512:1024, 512:1024] viewed [4, 128, 512]
                    nc.sync.dma_start(
                        out=out[512:, 512:].rearrange("(m p) n -> p m n", p=128),
                        in_=cphi.rearrange("p (m n) -> p m n", n=512),
                    )
                elif mb < 4 and nb == 1:
                    ms = slice(mb * 128, (mb + 1) * 128)
                    nc.sync.dma_start(out=out[ms, :], in_=cps[mb][:, :])
    nc.compile()
    return nc


def _host_prep(zipped_top, zipped_left, indicator, p):
    """Build fp8 pos operands and the packed bf16 aug tensor."""
    fp8 = ml_dtypes.float8_e4m3
    bf16 = ml_dtypes.bfloat16
    pos = {}
    s = {}
    for key, zipped in (("top", zipped_top), ("left", zipped_left)):
        b, seq, depth = zipped.shape
        oh = np.zeros((b, seq, TN + 1), dtype=np.float32)
        np.put_along_axis(oh, np.asarray(zipped, dtype=np.int64), 1.0, axis=2)
        oh = oh[..., :TN]
        s[key] = oh.sum(axis=2)                       # [b, seq]
        # posT [b, 512, seq] -> [b, 128, 4*seq] with k-tile blocks along free
        posT = oh.transpose(0, 2, 1).reshape(b, 4, 128, seq)
        pos[key] = np.ascontiguousarray(posT.transpose(0, 2, 1, 3)
                                        ).reshape(b, 128, 4 * seq)
    pad = (np.asarray(indicator) == 0).astype(np.float32)  # [b, seq]
    b, seq = pad.shape

    ins = {
        "lhs_top": (-2.0 * pos["top"]).astype(fp8),
        "rhs_top": pos["top"].astype(fp8),
        "lhs_left": (-2.0 * pos["left"]).astype(fp8),
        "rhs_left": pos["left"].astype(fp8),
    }
    augs = np.zeros((b, AUG, 4 * seq), dtype=np.float32)
    for mi, key in enumerate(("top", "left")):
        a = s[key] + p * pad
        lo, ro = (2 * mi) * seq, (2 * mi + 1) * seq
        augs[:, 0, lo:lo + seq] = a          # lhs row 0: A_i
        augs[:, 1, lo:lo + seq] = 1.0        # lhs row 1: ones
        augs[:, 2, lo:lo + seq] = pad        # lhs row 2: pad_i
        augs[:, 0, ro:ro + seq] = 1.0        # rhs row 0: ones
        augs[:, 1, ro:ro + seq] = a          # rhs row 1: B_j
        augs[:, 2, ro:ro + seq] = -p * pad   # rhs row 2: -p*pad_j
    ins["augs"] = augs.astype(bf16)
    return ins


def kernel(zipped_top, zipped_left, indicator, padding_dist):
    global LAST_RESULTS
    from concourse.bass_utils import run_bass_kernel_spmd

    p = float(np.asarray(padding_dist))
    ins = _host_prep(np.asarray(zipped_top), np.asarray(zipped_left),
                     indicator, p)

    if "nc" not in _NC_CACHE:
        _NC_CACHE["nc"] = _build_nc()
    nc = _NC_CACHE["nc"]

    in_maps = [{k: v[c] for k, v in ins.items()} for c in range(N_CORES)]
    res = run_bass_kernel_spmd(
        nc, in_maps, core_ids=list(range(N_CORES)),
        trace=os.environ.get("BASS_TRACE", "") == "1",
    )
    LAST_RESULTS = res
    full = np.stack([res.results[c]["out"] for c in range(N_CORES)]).astype(
        np.float32
    )
    # mirror the skipped below-diagonal blocks: rows 512:1024, cols 0:512
    full[:, 512:, :512] = full[:, :512, 512:].transpose(0, 2, 1)
    return full


# revision 15
# speedup vs baseline: 1.1442x; 1.1442x over previous
"""Sparse-attention distance-mask kernel for Trainium2 (8 NeuronCores).

Reference computation (per batch b):
    pos      = multi-hot of 4 tree-position ids over 512 nodes   [seq, 512]
    dist     = s_i + s_j - 2 * pos @ pos.T          (L1 dist of binary vecs)
    attn     = max(dist_top, dist_left)
    out      = attn + padding_dist * max(pad_i, pad_j)

Kernel strategy:
  - Data-parallel over batch: core c computes batch c (b == n_cores == 8).
  - The whole distance-with-padding map folds into ONE augmented GEMM per
    mask:  dist + pad_mat = (-2 pos_i).pos_j + rank-few augmentation rows
    carrying s_i, s_j and the padding terms.  All operand values are exact
    in fp8(e4m3) ({0,1,-2}, s<=4, and p=c1*c2 factor pairs), and PSUM
    accumulates fp32, so the result is bit-exact vs the f32 reference.
  - Per [128,512] output block: 4 fp8 K=128 passes + 1 aug pass (fp8 K=5
    when p factors exactly, else bf16 K=3).  DoubleRow measured slower here
    (N=512 moving stream dominates), so plain matmuls are used.
  - The distance map is symmetric: only 12 of 16 blocks are computed;
    below-diagonal blocks are mirrored on host.
  - Epilogue: ACT copies top-PSUM to SBUF, DVE maxes left-PSUM in, stores
    overlap compute (lower-right quadrant first, then row by row).
"""

import os

import ml_dtypes
import numpy as np

B, SEQ, DEPTH = 8, 1024, 4
TN = 512          # TOTAL_NODE
N_CORES = 8
MB, NB = SEQ // 128, SEQ // 512   # 8 x 2 grid of [128, 512] blocks

_NC_CACHE = {}
LAST_RESULTS = None

_POS_NAMES = ("lhs_top", "rhs_top", "lhs_left", "rhs_left")


def _build_nc(aug_rows, aug_dt_name):
    import concourse.mybir as mybir
    from concourse import bacc
    from concourse.tile import TileContext

    aug_dt = getattr(mybir.dt, aug_dt_name)
    nc = bacc.Bacc()
    dram = {}
    for name in _POS_NAMES:
        dram[name] = nc.dram_tensor(
            name, [128, 4 * SEQ], mybir.dt.float8e4, kind="ExternalInput"
        )
    dram["augs"] = nc.dram_tensor(
        "augs", [aug_rows, 4 * SEQ], aug_dt, kind="ExternalInput"
    )
    out = nc.dram_tensor("out", [SEQ, SEQ], mybir.dt.float32, kind="ExternalOutput")

    with TileContext(nc) as tc:
        with (
            tc.tile_pool(name="w", bufs=1) as wpool,
            tc.tile_pool(name="ps", bufs=2, space="PSUM") as ppool,
            tc.tile_pool(name="ep", bufs=1) as epool,
        ):
            sb = {}
            for name in _POS_NAMES:
                sb[name] = wpool.tile([128, 4 * SEQ], mybir.dt.float8e4,
                                      tag=name, name=name)
            augs = wpool.tile([aug_rows, 4 * SEQ], aug_dt,
                              tag="augs", name="augs")
            # 2 chunks per tensor for DMA-queue parallelism; top mask first
            for name in ("lhs_top", "rhs_top"):
                for c in range(2):
                    cs = slice(c * 2 * SEQ, (c + 1) * 2 * SEQ)
                    nc.sync.dma_start(out=sb[name][:, cs], in_=dram[name][:, cs])
            nc.sync.dma_start(out=augs[:, :], in_=dram["augs"][:, :])
            for name in ("lhs_left", "rhs_left"):
                for c in range(2):
                    cs = slice(c * 2 * SEQ, (c + 1) * 2 * SEQ)
                    nc.sync.dma_start(out=sb[name][:, cs], in_=dram[name][:, cs])

            # cp tiles: mb 0-3 hold a full [128,1024] row; mb 4-7 (upper
            # halves only) pack into one [128, 4*512] tile so the whole
            # lower-right quadrant stores with a single DMA.
            cps = {mb: epool.tile([128, SEQ], mybir.dt.float32,
                                  tag=f"cp{mb}", name=f"cp{mb}")
                   for mb in range(4)}
            cphi = epool.tile([128, 4 * 512], mybir.dt.float32,
                              tag="cphi", name="cphi")

            def cp_slice(mb, nb):
                if mb < 4:
                    return cps[mb][:, nb * 512:(nb + 1) * 512]
                return cphi[:, (mb - 4) * 512:(mb - 3) * 512]

            def gemm(psum, lname, rname, aug_l, aug_r, mb, nb):
                for kt in range(4):
                    nc.tensor.matmul(
                        psum[:, :],
                        lhsT=sb[lname][:, kt * SEQ + mb * 128:
                                       kt * SEQ + mb * 128 + 128],
                        rhs=sb[rname][:, kt * SEQ + nb * 512:
                                      kt * SEQ + nb * 512 + 512],
                        start=(kt == 0),
                        stop=False,
                    )
                nc.tensor.matmul(
                    psum[:, :],
                    lhsT=augs[:, aug_l * SEQ + mb * 128:
                              aug_l * SEQ + mb * 128 + 128],
                    rhs=augs[:, aug_r * SEQ + nb * 512:
                             aug_r * SEQ + nb * 512 + 512],
                    start=False,
                    stop=True,
                    skip_group_check=True,
                )

            # lower-right quadrant blocks first so their store DMA overlaps
            # the remaining compute; then full rows mb 0-3
            ordered = ([(mb, 1) for mb in range(4, 8)] +
                       [(mb, nb) for mb in range(4) for nb in range(2)])

            # Phase A: top-mask GEMMs -> ACT copy into cp
            for mb, nb in ordered:
                ps_t = ppool.tile([128, 512], mybir.dt.float32, tag="pt",
                                  name=f"pt{mb}_{nb}")
                gemm(ps_t, "lhs_top", "rhs_top", 0, 1, mb, nb)
                nc.scalar.copy(cp_slice(mb, nb), ps_t[:, :])

            # Phase B: left-mask GEMMs -> DVE max -> store (half-row DMAs
            # fire as soon as each block's max lands)
            for mb, nb in ordered:
                ps_l = ppool.tile([128, 512], mybir.dt.float32, tag="pl",
                                  name=f"pl{mb}_{nb}")
                gemm(ps_l, "lhs_left", "rhs_left", 2, 3, mb, nb)
                sl = cp_slice(mb, nb)
                nc.vector.tensor_max(sl, sl, ps_l[:, :])
                if mb == 7:
                    # one DMA for the whole lower-right quadrant:
                    # DRAM [512:1024, 512:1024] viewed [4, 128, 512]
                    nc.sync.dma_start(
                        out=out[512:, 512:].rearrange("(m p) n -> p m n", p=128),
                        in_=cphi.rearrange("p (m n) -> p m n", n=512),
                    )
                elif mb < 4:
                    ms = slice(mb * 128, (mb + 1) * 128)
                    ns = slice(nb * 512, (nb + 1) * 512)
                    nc.sync.dma_start(out=out[ms, ns], in_=sl)
    nc.compile()
    return nc


def _fp8_exact(x):
    f = x.astype(ml_dtypes.float8_e4m3).astype(np.float32)
    return np.array_equal(f, x)


def _aug_factor(p):
    """Find c1*c2 == p with c1, c2 fp8(e4m3)-exact; None if impossible."""
    for k in range(-6, 8):
        for m in range(8):
            c2 = np.float32(2.0 ** k) * np.float32(1 + m / 8.0)
            if c2 == 0:
                continue
            c1 = np.float32(p) / c2
            cand = np.array([c1, c2], dtype=np.float32)
            if c1 * c2 == np.float32(p) and _fp8_exact(cand):
                return float(c1), float(c2)
    return None


def _host_prep(zipped_top, zipped_left, indicator, p):
    """Build fp8 pos operands and the packed aug tensor."""
    fp8 = ml_dtypes.float8_e4m3
    pos = {}
    s = {}
    for key, zipped in (("top", zipped_top), ("left", zipped_left)):
        b, seq, depth = zipped.shape
        oh = np.zeros((b, seq, TN + 1), dtype=np.float32)
        np.put_along_axis(oh, np.asarray(zipped, dtype=np.int64), 1.0, axis=2)
        oh = oh[..., :TN]
        s[key] = oh.sum(axis=2)                       # [b, seq]
        # posT [b, 512, seq] -> [b, 128, 4*seq] with k-tile blocks along free
        posT = oh.transpose(0, 2, 1).reshape(b, 4, 128, seq)
        pos[key] = np.ascontiguousarray(posT.transpose(0, 2, 1, 3)
                                        ).reshape(b, 128, 4 * seq)
    pad = (np.asarray(indicator) == 0).astype(np.float32)  # [b, seq]
    b, seq = pad.shape

    ins = {
        "lhs_top": (-2.0 * pos["top"]).astype(fp8),
        "rhs_top": pos["top"].astype(fp8),
        "lhs_left": (-2.0 * pos["left"]).astype(fp8),
        "rhs_left": pos["left"].astype(fp8),
    }

    fac = _aug_factor(p)
    if fac is not None:
        # all-fp8 aug: 5 rank-1 terms
        c1, c2 = fac
        aug_rows, aug_np, aug_dt_name = 5, fp8, "float8e4"
        augs = np.zeros((b, 5, 4 * seq), dtype=np.float32)
        for mi, key in enumerate(("top", "left")):
            lo, ro = (2 * mi) * seq, (2 * mi + 1) * seq
            augs[:, 0, lo:lo + seq] = s[key]       # s_i * 1
            augs[:, 0, ro:ro + seq] = 1.0
            augs[:, 1, lo:lo + seq] = 1.0          # 1 * s_j
            augs[:, 1, ro:ro + seq] = s[key]
            augs[:, 2, lo:lo + seq] = c1 * pad     # p*pad_i
            augs[:, 2, ro:ro + seq] = c2
            augs[:, 3, lo:lo + seq] = c2           # p*pad_j
            augs[:, 3, ro:ro + seq] = c1 * pad
            augs[:, 4, lo:lo + seq] = c1 * pad     # -p*pad_i*pad_j
            augs[:, 4, ro:ro + seq] = -c2 * pad
    else:
        # bf16 fallback: 3 rows with A = s + p*pad
        aug_rows, aug_np, aug_dt_name = 3, ml_dtypes.bfloat16, "bfloat16"
        augs = np.zeros((b, 3, 4 * seq), dtype=np.float32)
        for mi, key in enumerate(("top", "left")):
            a = s[key] + p * pad
            lo, ro = (2 * mi) * seq, (2 * mi + 1) * seq
            augs[:, 0, lo:lo + seq] = a
            augs[:, 0, ro:ro + seq] = 1.0
            augs[:, 1, lo:lo + seq] = 1.0
            augs[:, 1, ro:ro + seq] = a
            augs[:, 2, lo:lo + seq] = pad
            augs[:, 2, ro:ro + seq] = -p * pad
    ins["augs"] = augs.astype(aug_np)
    return ins, aug_rows, aug_dt_name


def kernel(zipped_top, zipped_left, indicator, padding_dist):
    global LAST_RESULTS
    from concourse.bass_utils import run_bass_kernel_spmd

    p = float(np.asarray(padding_dist))
    ins, aug_rows, aug_dt_name = _host_prep(
        np.asarray(zipped_top), np.asarray(zipped_left), indicator, p)

    key = (aug_rows, aug_dt_name)
    if key not in _NC_CACHE:
        _NC_CACHE[key] = _build_nc(aug_rows, aug_dt_name)
    nc = _NC_CACHE[key]

    in_maps = [{k: v[c] for k, v in ins.items()} for c in range(N_CORES)]
    res = run_bass_kernel_spmd(
        nc, in_maps, core_ids=list(range(N_CORES)),
        trace=os.environ.get("BASS_TRACE", "") == "1",
    )
    LAST_RESULTS = res
    full = np.stack([res.results[c]["out"] for c in range(N_CORES)]).astype(
        np.float32
    )
    # mirror the skipped below-diagonal blocks: rows 512:1024, cols 0:512
    full[:, 512:, :512] = full[:, :512, 512:].transpose(0, 2, 1)
    return full
